# revision 33
# baseline (speedup 1.0000x reference)
"""Trainium2 Bass kernel for multi-head attention.

Problem: B=4, H=16, S=2048, D=128, fp32.
  scores = (q @ k^T) / sqrt(128); probs = softmax(scores, -1); out = probs @ v

Sharding: 64 (b,h) pairs -> 8 cores x 8 pairs. Fully independent per pair.

Per-(b,h) layout on device (everything "transposed", T-layout):
  qT, kT: [D=128, S=2048] in SBUF (host pre-transposes; contraction dim d on
  partitions).  For each t-tile (128 keys):
    scoresT[t, s] = kT[:, t-tile].T @ qT          (PE, fp32r)
    expT = exp(scoresT / sqrt(D))                 (ACT, fused scale, psum->sbuf)
    outT[d, s]  += v_tile[t, d].T-contraction     (PE: lhsT=v_tile, rhs=expT)
  softmax denominators (partition-dim sum of expT), split so neither PE
  nor DVE becomes the bottleneck:
    tiles 0..12:  acc += expT elementwise (DVE, chain finishes early),
    tiles 13..15: sums += ones.T @ expT  (PE; output rows replicated),
    then sums += ones.T @ acc            (PE, 2 matmuls).
  then out = outT * reciprocal(sums)     (DVE approx recip + mul)
Host transposes outT -> out per pair.

s is processed in halves of 1024 so PSUM fits in 8 banks: scores pool
(3 slots x 2 banks; one slot doubles as the sums accumulator during the
tail of each half) + outT accumulator (2 banks).  The per-tile loop is
software-pipelined by one tile so the PE never waits on ACT's exp
latency.  All matmul inputs are float32r (~13-bit mantissa, 1 cycle/row
at N=512 vs 4 for float32); end-to-end output error vs the fp32
reference is ~2e-4 relative.
"""

import sys

sys.path.insert(0, "/opt/trn_rl_repo")

import numpy as np

B, H, S, D = 4, 16, 2048, 128
N_CORES = 8
BH = B * H                      # 64 pairs
BH_PER_CORE = BH // N_CORES     # 8
T_TILES = S // 128              # 16
S_HALF = S // 2                 # 1024
SCALE = float(D) ** -0.5
PE_SUM_TILES = 3                # tiles summed via PE ones-matmul directly

_cache = {}


def _build_program():
    import concourse.tile as tile
    from concourse import bacc, mybir

    F32 = mybir.dt.float32
    F32R = mybir.dt.float32r

    nc = bacc.Bacc("TRN2", target_bir_lowering=False, debug=False)

    qt = nc.dram_tensor("qt", [BH_PER_CORE, D, S], F32R, kind="ExternalInput")
    kt = nc.dram_tensor("kt", [BH_PER_CORE, D, S], F32R, kind="ExternalInput")
    # v pre-shuffled on host to [p, t, d] so the load is fully contiguous
    v = nc.dram_tensor("v", [BH_PER_CORE, 128, T_TILES * D], F32R, kind="ExternalInput")
    ot = nc.dram_tensor("ot", [BH_PER_CORE, D, S], F32, kind="ExternalOutput")

    with tile.TileContext(nc) as tc:
        with (
            tc.tile_pool(name="const", bufs=1) as const,
            tc.tile_pool(name="rin", bufs=2) as rin,
            tc.tile_pool(name="exps", bufs=10) as exps,
            tc.tile_pool(name="accp", bufs=2) as accp,
            tc.tile_pool(name="outs", bufs=3) as outs,
            tc.tile_pool(name="psc", bufs=3, space="PSUM") as psc,
            tc.tile_pool(name="pacc", bufs=1, space="PSUM") as pacc,
        ):
            ones_f = const.tile([128, 128], F32)
            nc.vector.memset(ones_f[:], 1.0)
            ones_r = const.tile([128, 128], F32R)
            nc.vector.tensor_copy(ones_r[:], ones_f[:])

            for i in range(BH_PER_CORE):
                q_r = rin.tile([D, S], F32R, tag="q_r")
                k_r = rin.tile([D, S], F32R, tag="k_r")
                v_r = rin.tile([128, T_TILES, D], F32R, tag="v_r")
                # order: what the first tiles need comes first
                nc.sync.dma_start(out=k_r[:, :128], in_=kt[i, :, :128])
                nc.sync.dma_start(out=q_r[:, :512], in_=qt[i, :, :512])
                nc.sync.dma_start(out=k_r[:, 128:S_HALF], in_=kt[i, :, 128:S_HALF])
                nc.sync.dma_start(out=q_r[:, 512:S_HALF], in_=qt[i, :, 512:S_HALF])
                nc.sync.dma_start(
                    out=v_r[:], in_=v[i].rearrange("p (t d) -> p t d", t=T_TILES)
                )
                nc.sync.dma_start(out=q_r[:, S_HALF:], in_=qt[i, :, S_HALF:])
                nc.sync.dma_start(out=k_r[:, S_HALF:], in_=kt[i, :, S_HALF:])

                for h in range(2):
                    s0 = h * S_HALF
                    oacc = pacc.tile([128, S_HALF], F32, tag="oacc")
                    # sacc shares the scores-pool slots; only allocated for
                    # the tail of the half (tiles 13..15 + recip).
                    sacc_cell = [None]
                    eacc = accp.tile([128, S_HALF], F32R, tag="eacc")

                    ets = [None] * T_TILES

                    def pv(t):
                        for c in range(0, S_HALF, 512):
                            nc.tensor.matmul(
                                oacc[:, c : c + 512],
                                v_r[:, t, :],
                                ets[t][:, c : c + 512],
                                start=(t == 0),
                                stop=(t == T_TILES - 1),
                            )

                    dve_last = T_TILES - 1 - PE_SUM_TILES  # last tile in eacc

                    def consume(t):
                        pv(t)
                        if t == 1:
                            nc.vector.tensor_add(eacc[:], ets[0][:], ets[1][:])
                        elif 1 < t <= dve_last:
                            nc.vector.tensor_add(eacc[:], eacc[:], ets[t][:])
                        elif t > dve_last:
                            if t == dve_last + 1:
                                sacc_cell[0] = psc.tile(
                                    [128, S_HALF], F32, tag="sc", name="sacc"
                                )
                            sacc = sacc_cell[0]
                            for c in range(0, S_HALF, 512):
                                nc.tensor.matmul(
                                    sacc[:, c : c + 512],
                                    ones_r[:],
                                    ets[t][:, c : c + 512],
                                    start=(t == dve_last + 1),
                                    stop=False,
                                )
                            if t == T_TILES - 1:
                                # fold the DVE accumulator in last (its chain
                                # finished long ago; no PE wait here)
                                for c in range(0, S_HALF, 512):
                                    nc.tensor.matmul(
                                        sacc[:, c : c + 512],
                                        ones_r[:],
                                        eacc[:, c : c + 512],
                                        start=False,
                                        stop=True,
                                    )

                    for t in range(T_TILES):
                        sc = psc.tile([128, S_HALF], F32, tag="sc")
                        for c in range(0, S_HALF, 512):
                            nc.tensor.matmul(
                                sc[:, c : c + 512],
                                k_r[:, t * 128 : (t + 1) * 128],
                                q_r[:, s0 + c : s0 + c + 512],
                                start=True,
                                stop=True,
                            )
                        ets[t] = exps.tile(
                            [128, S_HALF], F32R, tag="et", name=f"et_{t}"
                        )
                        nc.scalar.activation(
                            ets[t][:],
                            sc[:],
                            mybir.ActivationFunctionType.Exp,
                            scale=SCALE,
                        )
                        if t >= 1:
                            consume(t - 1)
                    consume(T_TILES - 1)

                    rec = outs.tile([128, S_HALF], F32, tag="rec")
                    nc.vector.reciprocal_approx_fast(out=rec[:], in_=sacc_cell[0][:])
                    osb = outs.tile([128, S_HALF], F32, tag="osb")
                    nc.vector.tensor_mul(osb[:], oacc[:], rec[:])
                    nc.sync.dma_start(out=ot[i, :, s0 : s0 + S_HALF], in_=osb[:])

    nc.finalize()
    return nc


def _get_program():
    if "nc" not in _cache:
        _cache["nc"] = _build_program()
    return _cache["nc"]


def kernel(q: np.ndarray, k: np.ndarray, v: np.ndarray) -> np.ndarray:
    from concourse.bass_utils import run_bass_kernel_spmd

    nc = _get_program()

    q4 = np.ascontiguousarray(q, dtype=np.float32).reshape(BH, S, D)
    k4 = np.ascontiguousarray(k, dtype=np.float32).reshape(BH, S, D)
    v4 = np.ascontiguousarray(v, dtype=np.float32).reshape(BH, S, D)

    in_maps = []
    for core in range(N_CORES):
        sl = slice(core * BH_PER_CORE, (core + 1) * BH_PER_CORE)
        in_maps.append(
            {
                "qt": np.ascontiguousarray(q4[sl].transpose(0, 2, 1)),
                "kt": np.ascontiguousarray(k4[sl].transpose(0, 2, 1)),
                # [i, t*128+p, d] -> [i, p, t*128+d]
                "v": np.ascontiguousarray(
                    v4[sl]
                    .reshape(BH_PER_CORE, T_TILES, 128, D)
                    .transpose(0, 2, 1, 3)
                    .reshape(BH_PER_CORE, 128, T_TILES * D)
                ),
            }
        )

    res = run_bass_kernel_spmd(nc, in_maps, core_ids=list(range(N_CORES)))

    out = np.empty((BH, S, D), dtype=np.float32)
    for core in range(N_CORES):
        ot = res.results[core]["ot"]  # [BH_PER_CORE, D, S]
        out[core * BH_PER_CORE : (core + 1) * BH_PER_CORE] = ot.transpose(0, 2, 1)
    return out.reshape(B, H, S, D)


# revision 34
# speedup vs baseline: 1.0014x; 1.0014x over previous
"""Trainium2 Bass kernel for multi-head attention.

Problem: B=4, H=16, S=2048, D=128, fp32.
  scores = (q @ k^T) / sqrt(128); probs = softmax(scores, -1); out = probs @ v

Sharding: 64 (b,h) pairs -> 8 cores x 8 pairs. Fully independent per pair.

Per-(b,h) layout on device (everything "transposed", T-layout):
  qT, kT: [D=128, S=2048] in SBUF (host pre-transposes; contraction dim d on
  partitions).  For each t-tile (128 keys):
    scoresT[t, s] = kT[:, t-tile].T @ qT          (PE, fp32r)
    expT = exp(scoresT / sqrt(D))                 (ACT, fused scale, psum->sbuf)
    outT[d, s]  += v_tile[t, d].T-contraction     (PE: lhsT=v_tile, rhs=expT)
  softmax denominators (partition-dim sum of expT), split so neither PE
  nor DVE becomes the bottleneck:
    tiles 0..12:  acc += expT elementwise (DVE, chain finishes early),
    tiles 13..15: sums += ones.T @ expT  (PE; output rows replicated),
    then sums += ones.T @ acc            (PE, 2 matmuls).
  then out = outT * reciprocal(sums)     (DVE approx recip + mul)
Host transposes outT -> out per pair.

s is processed in halves of 1024 so PSUM fits in 8 banks: scores pool
(3 slots x 2 banks; one slot doubles as the sums accumulator during the
tail of each half) + outT accumulator (2 banks).  The per-tile loop is
software-pipelined by one tile so the PE never waits on ACT's exp
latency.  All matmul inputs are float32r (~13-bit mantissa, 1 cycle/row
at N=512 vs 4 for float32); end-to-end output error vs the fp32
reference is ~2e-4 relative.
"""

import sys

sys.path.insert(0, "/opt/trn_rl_repo")

import numpy as np

B, H, S, D = 4, 16, 2048, 128
N_CORES = 8
BH = B * H                      # 64 pairs
BH_PER_CORE = BH // N_CORES     # 8
T_TILES = S // 128              # 16
S_HALF = S // 2                 # 1024
SCALE = float(D) ** -0.5
PE_SUM_TILES = 3                # tiles summed via PE ones-matmul directly

_cache = {}


def _build_program():
    import concourse.tile as tile
    from concourse import bacc, mybir

    F32 = mybir.dt.float32
    F32R = mybir.dt.float32r

    nc = bacc.Bacc("TRN2", target_bir_lowering=False, debug=False)

    qt = nc.dram_tensor("qt", [BH_PER_CORE, D, S], F32R, kind="ExternalInput")
    kt = nc.dram_tensor("kt", [BH_PER_CORE, D, S], F32R, kind="ExternalInput")
    # v pre-shuffled on host to [p, t, d] so the load is fully contiguous
    v = nc.dram_tensor("v", [BH_PER_CORE, 128, T_TILES * D], F32R, kind="ExternalInput")
    ot = nc.dram_tensor("ot", [BH_PER_CORE, D, S], F32, kind="ExternalOutput")

    with tile.TileContext(nc) as tc:
        with (
            tc.tile_pool(name="const", bufs=1) as const,
            tc.tile_pool(name="rin", bufs=2) as rin,
            tc.tile_pool(name="exps", bufs=9) as exps,
            tc.tile_pool(name="accp", bufs=3) as accp,
            tc.tile_pool(name="outs", bufs=4) as outs,
            tc.tile_pool(name="psc", bufs=3, space="PSUM") as psc,
            tc.tile_pool(name="pacc", bufs=1, space="PSUM") as pacc,
        ):
            ones_f = const.tile([128, 128], F32)
            nc.vector.memset(ones_f[:], 1.0)
            ones_r = const.tile([128, 128], F32R)
            nc.vector.tensor_copy(ones_r[:], ones_f[:])

            for i in range(BH_PER_CORE):
                q_r = rin.tile([D, S], F32R, tag="q_r")
                k_r = rin.tile([D, S], F32R, tag="k_r")
                v_r = rin.tile([128, T_TILES, D], F32R, tag="v_r")
                # order: what the first tiles need comes first
                nc.sync.dma_start(out=k_r[:, :128], in_=kt[i, :, :128])
                nc.sync.dma_start(out=q_r[:, :512], in_=qt[i, :, :512])
                nc.sync.dma_start(out=k_r[:, 128:S_HALF], in_=kt[i, :, 128:S_HALF])
                nc.sync.dma_start(out=q_r[:, 512:S_HALF], in_=qt[i, :, 512:S_HALF])
                nc.sync.dma_start(
                    out=v_r[:], in_=v[i].rearrange("p (t d) -> p t d", t=T_TILES)
                )
                nc.sync.dma_start(out=q_r[:, S_HALF:], in_=qt[i, :, S_HALF:])
                nc.sync.dma_start(out=k_r[:, S_HALF:], in_=kt[i, :, S_HALF:])

                for h in range(2):
                    s0 = h * S_HALF
                    oacc = pacc.tile([128, S_HALF], F32, tag="oacc")
                    # sacc shares the scores-pool slots; only allocated for
                    # the tail of the half (tiles 13..15 + recip).
                    sacc_cell = [None]
                    eacc = accp.tile([128, S_HALF], F32R, tag="eacc")

                    ets = [None] * T_TILES

                    def pv(t):
                        for c in range(0, S_HALF, 512):
                            nc.tensor.matmul(
                                oacc[:, c : c + 512],
                                v_r[:, t, :],
                                ets[t][:, c : c + 512],
                                start=(t == 0),
                                stop=(t == T_TILES - 1),
                            )

                    dve_last = T_TILES - 1 - PE_SUM_TILES  # last tile in eacc

                    def consume(t):
                        pv(t)
                        if t == 1:
                            nc.vector.tensor_add(eacc[:], ets[0][:], ets[1][:])
                        elif 1 < t <= dve_last:
                            nc.vector.tensor_add(eacc[:], eacc[:], ets[t][:])
                        elif t > dve_last:
                            if t == dve_last + 1:
                                sacc_cell[0] = psc.tile(
                                    [128, S_HALF], F32, tag="sc", name="sacc"
                                )
                            sacc = sacc_cell[0]
                            for c in range(0, S_HALF, 512):
                                nc.tensor.matmul(
                                    sacc[:, c : c + 512],
                                    ones_r[:],
                                    ets[t][:, c : c + 512],
                                    start=(t == dve_last + 1),
                                    stop=False,
                                )
                            if t == T_TILES - 1:
                                # fold the DVE accumulator in last (its chain
                                # finished long ago; no PE wait here)
                                for c in range(0, S_HALF, 512):
                                    nc.tensor.matmul(
                                        sacc[:, c : c + 512],
                                        ones_r[:],
                                        eacc[:, c : c + 512],
                                        start=False,
                                        stop=True,
                                    )

                    for t in range(T_TILES):
                        sc = psc.tile([128, S_HALF], F32, tag="sc")
                        for c in range(0, S_HALF, 512):
                            nc.tensor.matmul(
                                sc[:, c : c + 512],
                                k_r[:, t * 128 : (t + 1) * 128],
                                q_r[:, s0 + c : s0 + c + 512],
                                start=True,
                                stop=True,
                            )
                        ets[t] = exps.tile(
                            [128, S_HALF], F32R, tag="et", name=f"et_{t}"
                        )
                        nc.scalar.activation(
                            ets[t][:],
                            sc[:],
                            mybir.ActivationFunctionType.Exp,
                            scale=SCALE,
                        )
                        if t >= 1:
                            consume(t - 1)
                    consume(T_TILES - 1)

                    rec = outs.tile([128, S_HALF], F32, tag="rec")
                    nc.vector.reciprocal_approx_fast(out=rec[:], in_=sacc_cell[0][:])
                    osb = outs.tile([128, S_HALF], F32, tag="osb")
                    nc.vector.tensor_mul(osb[:], oacc[:], rec[:])
                    nc.sync.dma_start(out=ot[i, :, s0 : s0 + S_HALF], in_=osb[:])

    nc.finalize()
    return nc


def _get_program():
    if "nc" not in _cache:
        _cache["nc"] = _build_program()
    return _cache["nc"]


def kernel(q: np.ndarray, k: np.ndarray, v: np.ndarray) -> np.ndarray:
    from concourse.bass_utils import run_bass_kernel_spmd

    nc = _get_program()

    q4 = np.ascontiguousarray(q, dtype=np.float32).reshape(BH, S, D)
    k4 = np.ascontiguousarray(k, dtype=np.float32).reshape(BH, S, D)
    v4 = np.ascontiguousarray(v, dtype=np.float32).reshape(BH, S, D)

    in_maps = []
    for core in range(N_CORES):
        sl = slice(core * BH_PER_CORE, (core + 1) * BH_PER_CORE)
        in_maps.append(
            {
                "qt": np.ascontiguousarray(q4[sl].transpose(0, 2, 1)),
                "kt": np.ascontiguousarray(k4[sl].transpose(0, 2, 1)),
                # [i, t*128+p, d] -> [i, p, t*128+d]
                "v": np.ascontiguousarray(
                    v4[sl]
                    .reshape(BH_PER_CORE, T_TILES, 128, D)
                    .transpose(0, 2, 1, 3)
                    .reshape(BH_PER_CORE, 128, T_TILES * D)
                ),
            }
        )

    res = run_bass_kernel_spmd(nc, in_maps, core_ids=list(range(N_CORES)))

    out = np.empty((BH, S, D), dtype=np.float32)
    for core in range(N_CORES):
        ot = res.results[core]["ot"]  # [BH_PER_CORE, D, S]
        out[core * BH_PER_CORE : (core + 1) * BH_PER_CORE] = ot.transpose(0, 2, 1)
    return out.reshape(B, H, S, D)


# revision 35
# speedup vs baseline: 1.0027x; 1.0013x over previous
"""Trainium2 Bass kernel for multi-head attention.

Problem: B=4, H=16, S=2048, D=128, fp32.
  scores = (q @ k^T) / sqrt(128); probs = softmax(scores, -1); out = probs @ v

Sharding: 64 (b,h) pairs -> 8 cores x 8 pairs. Fully independent per pair.

Per-(b,h) layout on device (everything "transposed", T-layout):
  qT, kT: [D=128, S=2048] in SBUF (host pre-transposes; contraction dim d on
  partitions).  For each t-tile (128 keys):
    scoresT[t, s] = kT[:, t-tile].T @ qT          (PE, fp32r)
    expT = exp(scoresT / sqrt(D))                 (ACT, fused scale, psum->sbuf)
    outT[d, s]  += v_tile[t, d].T-contraction     (PE: lhsT=v_tile, rhs=expT)
  softmax denominators (partition-dim sum of expT), split so neither PE
  nor DVE becomes the bottleneck:
    tiles 0..12:  acc += expT elementwise (DVE, chain finishes early),
    tiles 13..15: sums += ones.T @ expT  (PE; output rows replicated),
    then sums += ones.T @ acc            (PE, 2 matmuls).
  then out = outT * reciprocal(sums)     (DVE approx recip + mul)
Host transposes outT -> out per pair.

s is processed in halves of 1024 so PSUM fits in 8 banks: scores pool
(3 slots x 2 banks; one slot doubles as the sums accumulator during the
tail of each half) + outT accumulator (2 banks).  The per-tile loop is
software-pipelined by one tile so the PE never waits on ACT's exp
latency.  All matmul inputs are float32r (~13-bit mantissa, 1 cycle/row
at N=512 vs 4 for float32); end-to-end output error vs the fp32
reference is ~2e-4 relative.
"""

import sys

sys.path.insert(0, "/opt/trn_rl_repo")

import numpy as np

B, H, S, D = 4, 16, 2048, 128
N_CORES = 8
BH = B * H                      # 64 pairs
BH_PER_CORE = BH // N_CORES     # 8
T_TILES = S // 128              # 16
S_HALF = S // 2                 # 1024
SCALE = float(D) ** -0.5
PE_SUM_TILES = 3                # tiles summed via PE ones-matmul directly

_cache = {}


def _build_program():
    import concourse.tile as tile
    from concourse import bacc, mybir

    F32 = mybir.dt.float32
    F32R = mybir.dt.float32r

    nc = bacc.Bacc("TRN2", target_bir_lowering=False, debug=False)

    qt = nc.dram_tensor("qt", [BH_PER_CORE, D, S], F32R, kind="ExternalInput")
    kt = nc.dram_tensor("kt", [BH_PER_CORE, D, S], F32R, kind="ExternalInput")
    # v pre-shuffled on host to [p, t, d] so the load is fully contiguous
    v = nc.dram_tensor("v", [BH_PER_CORE, 128, T_TILES * D], F32R, kind="ExternalInput")
    ot = nc.dram_tensor("ot", [BH_PER_CORE, D, S], F32, kind="ExternalOutput")

    with tile.TileContext(nc) as tc:
        with (
            tc.tile_pool(name="const", bufs=1) as const,
            tc.tile_pool(name="rin", bufs=2) as rin,
            tc.tile_pool(name="exps", bufs=10) as exps,
            tc.tile_pool(name="accp", bufs=3) as accp,
            tc.tile_pool(name="outs", bufs=4) as outs,
            tc.tile_pool(name="psc", bufs=3, space="PSUM") as psc,
            tc.tile_pool(name="pacc", bufs=1, space="PSUM") as pacc,
        ):
            ones_f = const.tile([128, 128], F32)
            nc.vector.memset(ones_f[:], 1.0)
            ones_r = const.tile([128, 128], F32R)
            nc.vector.tensor_copy(ones_r[:], ones_f[:])

            for i in range(BH_PER_CORE):
                q_r = rin.tile([D, S], F32R, tag="q_r")
                k_r = rin.tile([D, S], F32R, tag="k_r")
                v_r = rin.tile([128, T_TILES, D], F32R, tag="v_r")
                # order: what the first tiles need comes first
                nc.sync.dma_start(out=k_r[:, :128], in_=kt[i, :, :128])
                nc.sync.dma_start(out=q_r[:, :512], in_=qt[i, :, :512])
                nc.sync.dma_start(out=k_r[:, 128:S_HALF], in_=kt[i, :, 128:S_HALF])
                nc.sync.dma_start(out=q_r[:, 512:S_HALF], in_=qt[i, :, 512:S_HALF])
                nc.sync.dma_start(
                    out=v_r[:], in_=v[i].rearrange("p (t d) -> p t d", t=T_TILES)
                )
                nc.sync.dma_start(out=q_r[:, S_HALF:], in_=qt[i, :, S_HALF:])
                nc.sync.dma_start(out=k_r[:, S_HALF:], in_=kt[i, :, S_HALF:])

                for h in range(2):
                    s0 = h * S_HALF
                    oacc = pacc.tile([128, S_HALF], F32, tag="oacc")
                    # sacc shares the scores-pool slots; only allocated for
                    # the tail of the half (tiles 13..15 + recip).
                    sacc_cell = [None]
                    eacc = accp.tile([128, S_HALF], F32R, tag="eacc")

                    ets = [None] * T_TILES

                    def pv(t):
                        for c in range(0, S_HALF, 512):
                            nc.tensor.matmul(
                                oacc[:, c : c + 512],
                                v_r[:, t, :],
                                ets[t][:, c : c + 512],
                                start=(t == 0),
                                stop=(t == T_TILES - 1),
                            )

                    dve_last = T_TILES - 1 - PE_SUM_TILES  # last tile in eacc

                    def consume(t):
                        pv(t)
                        if t == 1:
                            nc.vector.tensor_add(eacc[:], ets[0][:], ets[1][:])
                        elif 1 < t <= dve_last:
                            nc.vector.tensor_add(eacc[:], eacc[:], ets[t][:])
                        elif t > dve_last:
                            if t == dve_last + 1:
                                sacc_cell[0] = psc.tile(
                                    [128, S_HALF], F32, tag="sc", name="sacc"
                                )
                            sacc = sacc_cell[0]
                            for c in range(0, S_HALF, 512):
                                nc.tensor.matmul(
                                    sacc[:, c : c + 512],
                                    ones_r[:],
                                    ets[t][:, c : c + 512],
                                    start=(t == dve_last + 1),
                                    stop=False,
                                )
                            if t == T_TILES - 1:
                                # fold the DVE accumulator in last (its chain
                                # finished long ago; no PE wait here)
                                for c in range(0, S_HALF, 512):
                                    nc.tensor.matmul(
                                        sacc[:, c : c + 512],
                                        ones_r[:],
                                        eacc[:, c : c + 512],
                                        start=False,
                                        stop=True,
                                    )

                    for t in range(T_TILES):
                        sc = psc.tile([128, S_HALF], F32, tag="sc")
                        for c in range(0, S_HALF, 512):
                            nc.tensor.matmul(
                                sc[:, c : c + 512],
                                k_r[:, t * 128 : (t + 1) * 128],
                                q_r[:, s0 + c : s0 + c + 512],
                                start=True,
                                stop=True,
                            )
                        ets[t] = exps.tile(
                            [128, S_HALF], F32R, tag="et", name=f"et_{t}"
                        )
                        nc.scalar.activation(
                            ets[t][:],
                            sc[:],
                            mybir.ActivationFunctionType.Exp,
                            scale=SCALE,
                        )
                        if t >= 1:
                            consume(t - 1)
                    consume(T_TILES - 1)

                    rec = outs.tile([128, S_HALF], F32, tag="rec")
                    nc.vector.reciprocal_approx_fast(out=rec[:], in_=sacc_cell[0][:])
                    osb = outs.tile([128, S_HALF], F32, tag="osb")
                    nc.vector.tensor_mul(osb[:], oacc[:], rec[:])
                    nc.sync.dma_start(out=ot[i, :, s0 : s0 + S_HALF], in_=osb[:])

    nc.finalize()
    return nc


def _get_program():
    if "nc" not in _cache:
        _cache["nc"] = _build_program()
    return _cache["nc"]


def kernel(q: np.ndarray, k: np.ndarray, v: np.ndarray) -> np.ndarray:
    from concourse.bass_utils import run_bass_kernel_spmd

    nc = _get_program()

    q4 = np.ascontiguousarray(q, dtype=np.float32).reshape(BH, S, D)
    k4 = np.ascontiguousarray(k, dtype=np.float32).reshape(BH, S, D)
    v4 = np.ascontiguousarray(v, dtype=np.float32).reshape(BH, S, D)

    in_maps = []
    for core in range(N_CORES):
        sl = slice(core * BH_PER_CORE, (core + 1) * BH_PER_CORE)
        in_maps.append(
            {
                "qt": np.ascontiguousarray(q4[sl].transpose(0, 2, 1)),
                "kt": np.ascontiguousarray(k4[sl].transpose(0, 2, 1)),
                # [i, t*128+p, d] -> [i, p, t*128+d]
                "v": np.ascontiguousarray(
                    v4[sl]
                    .reshape(BH_PER_CORE, T_TILES, 128, D)
                    .transpose(0, 2, 1, 3)
                    .reshape(BH_PER_CORE, 128, T_TILES * D)
                ),
            }
        )

    res = run_bass_kernel_spmd(nc, in_maps, core_ids=list(range(N_CORES)))

    out = np.empty((BH, S, D), dtype=np.float32)
    for core in range(N_CORES):
        ot = res.results[core]["ot"]  # [BH_PER_CORE, D, S]
        out[core * BH_PER_CORE : (core + 1) * BH_PER_CORE] = ot.transpose(0, 2, 1)
    return out.reshape(B, H, S, D)
